# revision 4
# baseline (speedup 1.0000x reference)
"""Conv2D 3x3 (stride 1, pad 1) NCHW kernel for Trainium2, 8 NeuronCores.

Reference op: y = conv2d(x[32,128,56,56], w[256,128,3,3]) + b[256]  (fp32)

Strategy:
  - Data-parallel over batch: 4 images per core, 8 cores.
  - Conv as 9 shifted matmuls accumulating in PSUM:
      out[co, (h,w)] = sum_{kh,kw} W[kh,kw][ci,co].T @ xpad[ci, h+kh, w+kw]
    K = ci = 128 sits exactly on the 128 SBUF partitions.
  - Host pre-pads x to 58x58 (zero halo) and pre-transposes the weight to
    [ci, (kh kw), co], so the device never transposes or memsets anything.
  - fp16 operands (host-cast): full PE rate (1 col/cycle), separate
    LDWEIGHTS with FWL that the PE's reorder window hides behind matmuls.
  - One [128, 7, 512] PSUM tile spans 7 banks: each 8-row output block
    accumulates in its own bank; evictions are batched as two multi-bank
    activations per (co-chunk, image) -- banks 0-3 then 4-6 -- cutting the
    per-ACTIVATE ~352-cycle fixed cost ~4x vs per-bank eviction.
  - Evictions land in a [128, 3136] SBUF staging tile; the whole
    (co-chunk, image) output leaves in ONE 1.6MB (fp32) DMA instead of 7
    small ones (measured ~8us/rep of per-store overhead at 56 stores/rep).
  - Timing loop (reps>1) is unrolled 2x with double-buffered x tiles so
    the next rep's x load overlaps the current rep's matmuls.
"""

import numpy as np

N_CORES = 8
B, CI, H, W = 32, 128, 56, 56
CO = 256
KH = KW = 3
NTAPS = KH * KW
BS = B // N_CORES            # images per core
HP, WP = H + 2, W + 2        # padded image
HB = 8                       # output rows per block
NB = H // HB                 # blocks per image
NTILE = HB * W               # 448 fp32 per PSUM bank (of 512)
NCHUNK = CO // 128           # co chunks of 128 partitions

WDTYPE = "float16"           # matmul operand dtype (host-cast)
STORE16 = False              # store y as fp16, upcast on host

_cache = {}


def _build(reps=1, wdtype=WDTYPE, internal_io=False, store16=STORE16):
    """Build + compile the SPMD program.

    reps>1 wraps the body in a For_i hardware loop, unrolled 2x with
    double-buffered x tiles (used for timing). internal_io keeps x/y in
    device DRAM with a small token output (timing-only: no host transfer
    of the big tensors).
    """
    import concourse.mybir as mybir
    import concourse.tile as tile
    from concourse import bacc

    mmdt = getattr(mybir.dt, wdtype)
    ydt = mybir.dt.float16 if store16 else mybir.dt.float32

    nc = bacc.Bacc("TRN2", target_bir_lowering=False, debug=False)
    if internal_io:
        xp_ap = nc.dram_tensor("xp_i", [BS, CI, HP, WP], mmdt).ap()
        y_ap = nc.dram_tensor("y_i", [BS, CO, H, W], ydt).ap()
        tok_ap = nc.dram_tensor(
            "tok", [128, NCHUNK], mybir.dt.float32, kind="ExternalOutput"
        ).ap()
    else:
        xp_ap = nc.dram_tensor(
            "xp", [BS, CI, HP, WP], mmdt, kind="ExternalInput"
        ).ap()
        y_ap = nc.dram_tensor(
            "y", [BS, CO, H, W], ydt, kind="ExternalOutput"
        ).ap()
        tok_ap = None
    wt_ap = nc.dram_tensor(
        "wt", [CI, NTAPS * CO], mmdt, kind="ExternalInput"
    ).ap()
    bt_ap = nc.dram_tensor(
        "bt", [128, NCHUNK], mybir.dt.float32, kind="ExternalInput"
    ).ap()

    with tile.TileContext(nc) as tc:
        with (
            tc.tile_pool(name="xw", bufs=1) as xw,
            tc.tile_pool(name="out", bufs=3) as outp,
            tc.tile_pool(name="ps", bufs=1, space="PSUM") as ps,
        ):
            wsb = xw.tile([CI, NTAPS * CO], mmdt, tag="w")
            bsb = xw.tile([128, NCHUNK], mybir.dt.float32, tag="b")
            nc.scalar.dma_start(out=wsb[:], in_=wt_ap[:, :])
            nc.scalar.dma_start(out=bsb[:], in_=bt_ap[:, :])

            # 7 PSUM banks; block hb accumulates in bank hb (first 448 of 512)
            psb = ps.tile([128, NB, 512], mybir.dt.float32, tag="acc")

            def body():
                xsb = xw.tile([CI, BS * HP * WP], mmdt, tag="x", bufs=2)
                xdma = nc.gpsimd
                # priority chunk: rows 0..HB+1 of img0 — everything the first
                # matmul group reads — so PE can start ~1us in on a cold start
                head = (HB + 2) * WP
                xflat0 = xp_ap[0].rearrange("c h w -> c (h w)")
                xdma.dma_start(out=xsb[:, 0:head], in_=xflat0[:, 0:head])
                xdma.dma_start(
                    out=xsb[:, head : HP * WP], in_=xflat0[:, head : HP * WP]
                )
                for img in range(1, BS):
                    xdma.dma_start(
                        out=xsb[:, img * HP * WP : (img + 1) * HP * WP],
                        in_=xp_ap[img].rearrange("c h w -> c (h w)")[:, :],
                    )
                xv = xsb[:].rearrange("c (n h w) -> c n h w", n=BS, h=HP)

                for c in range(NCHUNK):
                    for img in range(BS):
                        ob = outp.tile([128, NB * NTILE], ydt, tag="o")
                        for hb in range(NB):
                            pt = psb[:, hb, 0:NTILE]
                            for kh in range(KH):
                                for kw in range(KW):
                                    tap = kh * KW + kw
                                    r0 = hb * HB + kh
                                    nc.tensor.matmul(
                                        pt,
                                        wsb[
                                            :,
                                            tap * CO
                                            + c * 128 : tap * CO
                                            + (c + 1) * 128,
                                        ],
                                        xv[:, img, r0 : r0 + HB, kw : kw + W],
                                        start=(tap == 0),
                                        stop=(tap == NTAPS - 1),
                                    )
                            if hb == 3:
                                # banks 0-3 done: evict while PE fills 4-6
                                nc.scalar.activation(
                                    ob[:, 0 : 4 * NTILE].rearrange(
                                        "c (a b) -> c a b", a=4
                                    ),
                                    psb[:, 0:4, 0:NTILE],
                                    mybir.ActivationFunctionType.Identity,
                                    bias=bsb[:, c : c + 1],
                                    scale=1.0,
                                )
                        nc.scalar.activation(
                            ob[:, 4 * NTILE : NB * NTILE].rearrange(
                                "c (a b) -> c a b", a=NB - 4
                            ),
                            psb[:, 4:NB, 0:NTILE],
                            mybir.ActivationFunctionType.Identity,
                            bias=bsb[:, c : c + 1],
                            scale=1.0,
                        )
                        nc.sync.dma_start(
                            out=y_ap[img, c * 128 : (c + 1) * 128],
                            in_=ob[:],
                        )

            if reps > 1:
                with tc.For_i(0, reps // 2, 1, hint_engines=(mybir.EngineType.PE,)):
                    body()
                    body()
            else:
                body()
            if tok_ap is not None:
                nc.sync.dma_start(out=tok_ap[:, :], in_=bsb[:])
    nc.compile()
    return nc


def _get_nc(reps=1, wdtype=WDTYPE, internal_io=False, store16=STORE16):
    key = (reps, wdtype, internal_io, store16)
    if key not in _cache:
        _cache[key] = _build(reps, wdtype, internal_io, store16)
    return _cache[key]


def _prep_inputs(x, weight, bias, wdtype=WDTYPE):
    npdt = np.float16 if wdtype == "float16" else np.float32
    x = np.asarray(x)
    weight = np.ascontiguousarray(weight, dtype=np.float32)
    bias = np.ascontiguousarray(bias, dtype=np.float32)
    # fused pad+cast: one pass over x instead of pad(fp32) then astype
    xpad = np.zeros((B, CI, HP, WP), dtype=npdt)
    xpad[:, :, 1 : H + 1, 1 : W + 1] = x
    # [co, ci, kh, kw] -> [ci, (kh kw), co] flattened to [ci, 9*co]
    wt = np.ascontiguousarray(
        weight.transpose(1, 2, 3, 0).reshape(CI, NTAPS * CO).astype(npdt)
    )
    bt = np.ascontiguousarray(bias.reshape(NCHUNK, 128).T)
    in_maps = [
        {
            "xp": np.ascontiguousarray(xpad[i * BS : (i + 1) * BS]),
            "wt": wt,
            "bt": bt,
        }
        for i in range(N_CORES)
    ]
    return in_maps


def run_sharded(x, weight, bias, trace=False, reps=1, wdtype=WDTYPE):
    """Run on all 8 cores; returns (full_output, BassKernelResults)."""
    from concourse.bass_utils import run_bass_kernel_spmd

    nc = _get_nc(reps, wdtype)
    in_maps = _prep_inputs(x, weight, bias, wdtype)
    res = run_bass_kernel_spmd(nc, in_maps, list(range(N_CORES)), trace=trace)
    y = np.concatenate([res.results[i]["y"] for i in range(N_CORES)], axis=0)
    if y.dtype != np.float32:
        y = y.astype(np.float32)
    return y, res


def kernel(x, weight, bias):
    y, _ = run_sharded(x, weight, bias)
    return y


# revision 7
# speedup vs baseline: 1.3757x; 1.3757x over previous
"""Conv2D 3x3 (stride 1, pad 1) NCHW kernel for Trainium2, 8 NeuronCores.

Reference op: y = conv2d(x[32,128,56,56], w[256,128,3,3]) + b[256]  (fp32)

Strategy:
  - Data-parallel over batch: 4 images per core, 8 cores.
  - Conv as 9 shifted matmuls accumulating in PSUM:
      out[co, (h,w)] = sum_{kh,kw} W[kh,kw][ci,co].T @ xpad[ci, h+kh, w+kw]
    K = ci = 128 sits exactly on the 128 SBUF partitions.
  - Host pre-pads x to 58x58 (zero halo) and pre-transposes the weight to
    [ci, (kh kw), co], so the device never transposes or memsets anything.
  - fp16 operands (host-cast): full PE rate (1 col/cycle), separate
    LDWEIGHTS with FWL that the PE's reorder window hides behind matmuls.
  - Each 8-row output block accumulates in its own PSUM bank (pool bufs=7);
    evictions (bias fused, scalar engine) land in a [128, 3136] SBUF staging
    tile, and the whole (co-chunk, image) output leaves in ONE DMA instead
    of 7 small ones (saves ~8us/rep of per-store overhead at 56 stores/rep).
    Output is staged/stored as fp16 (halves store traffic) and upcast to
    fp32 on the host: adds ~4e-4 max-rel error on top of the ~2e-4 fp16
    matmul error, well inside the 2e-3 gate.
  - Timing loop (reps>1) is unrolled 2x with double-buffered x tiles so
    the next rep's x load overlaps the current rep's matmuls.
"""

import numpy as np

N_CORES = 8
B, CI, H, W = 32, 128, 56, 56
CO = 256
KH = KW = 3
NTAPS = KH * KW
BS = B // N_CORES            # images per core
HP, WP = H + 2, W + 2        # padded image
HB = 8                       # output rows per block
NB = H // HB                 # blocks per image
NTILE = HB * W               # 448 fp32 -> one PSUM bank
NCHUNK = CO // 128           # co chunks of 128 partitions

WDTYPE = "float16"           # matmul operand dtype (host-cast)
STORE16 = True               # store y as fp16, upcast on host (halves store
                             # traffic; adds ~4e-4 max-rel error, well under
                             # the 2e-3 gate)
EVICT = "act"                # "act" | "dve" | "both" (alternate)
BATCH_STORE = True           # stage NB blocks in SBUF, one store per (c,img)

_cache = {}


def _build(reps=1, wdtype=WDTYPE, internal_io=False, store16=STORE16,
           evict=None, batch_store=None):
    """Build + compile the SPMD program."""
    import concourse.mybir as mybir
    import concourse.tile as tile
    from concourse import bacc

    evict = EVICT if evict is None else evict
    batch_store = BATCH_STORE if batch_store is None else batch_store

    mmdt = getattr(mybir.dt, wdtype)
    ydt = mybir.dt.float16 if store16 else mybir.dt.float32

    nc = bacc.Bacc("TRN2", target_bir_lowering=False, debug=False)
    if internal_io:
        xp_ap = nc.dram_tensor("xp_i", [BS, CI, HP, WP], mmdt).ap()
        y_ap = nc.dram_tensor("y_i", [BS, CO, H, W], ydt).ap()
        tok_ap = nc.dram_tensor(
            "tok", [128, NCHUNK], mybir.dt.float32, kind="ExternalOutput"
        ).ap()
    else:
        xp_ap = nc.dram_tensor(
            "xp", [BS, CI, HP, WP], mmdt, kind="ExternalInput"
        ).ap()
        y_ap = nc.dram_tensor(
            "y", [BS, CO, H, W], ydt, kind="ExternalOutput"
        ).ap()
        tok_ap = None
    wt_ap = nc.dram_tensor(
        "wt", [CI, NTAPS * CO], mmdt, kind="ExternalInput"
    ).ap()
    bt_ap = nc.dram_tensor(
        "bt", [128, NCHUNK], mybir.dt.float32, kind="ExternalInput"
    ).ap()

    with tile.TileContext(nc) as tc:
        with (
            tc.tile_pool(name="xw", bufs=1) as xw,
            tc.tile_pool(name="out", bufs=3 if batch_store else 8) as outp,
            tc.tile_pool(name="ps", bufs=7, space="PSUM") as ps,
        ):
            wsb = xw.tile([CI, NTAPS * CO], mmdt, tag="w")
            bsb = xw.tile([128, NCHUNK], mybir.dt.float32, tag="b")
            nc.scalar.dma_start(out=wsb[:], in_=wt_ap[:, :])
            nc.scalar.dma_start(out=bsb[:], in_=bt_ap[:, :])

            def evict_one(dst, pt, c, k):
                use = evict if evict != "both" else ("act" if k % 2 else "dve")
                if use == "act":
                    nc.scalar.activation(
                        dst,
                        pt,
                        mybir.ActivationFunctionType.Identity,
                        bias=bsb[:, c : c + 1],
                        scale=1.0,
                    )
                else:
                    nc.vector.tensor_scalar_add(dst, pt, bsb[:, c : c + 1])

            def body():
                xsb = xw.tile([CI, BS * HP * WP], mmdt, tag="x", bufs=2)
                xdma = nc.gpsimd
                head = (HB + 2) * WP
                xflat0 = xp_ap[0].rearrange("c h w -> c (h w)")
                xdma.dma_start(out=xsb[:, 0:head], in_=xflat0[:, 0:head])
                xdma.dma_start(
                    out=xsb[:, head : HP * WP], in_=xflat0[:, head : HP * WP]
                )
                for img in range(1, BS):
                    xdma.dma_start(
                        out=xsb[:, img * HP * WP : (img + 1) * HP * WP],
                        in_=xp_ap[img].rearrange("c h w -> c (h w)")[:, :],
                    )
                xv = xsb[:].rearrange("c (n h w) -> c n h w", n=BS, h=HP)

                k = 0
                for c in range(NCHUNK):
                    for img in range(BS):
                        if batch_store:
                            ob = outp.tile([128, NB * NTILE], ydt, tag="o")
                        for hb in range(NB):
                            pt = ps.tile([128, NTILE], mybir.dt.float32, tag="acc")
                            for kh in range(KH):
                                for kw in range(KW):
                                    tap = kh * KW + kw
                                    r0 = hb * HB + kh
                                    nc.tensor.matmul(
                                        pt[:],
                                        wsb[
                                            :,
                                            tap * CO
                                            + c * 128 : tap * CO
                                            + (c + 1) * 128,
                                        ],
                                        xv[:, img, r0 : r0 + HB, kw : kw + W],
                                        start=(tap == 0),
                                        stop=(tap == NTAPS - 1),
                                    )
                            if batch_store:
                                evict_one(
                                    ob[:, hb * NTILE : (hb + 1) * NTILE],
                                    pt[:], c, k,
                                )
                            else:
                                ot = outp.tile([128, NTILE], ydt, tag="o")
                                evict_one(ot[:], pt[:], c, k)
                                nc.sync.dma_start(
                                    out=y_ap[
                                        img,
                                        c * 128 : (c + 1) * 128,
                                        hb * HB : (hb + 1) * HB,
                                        :,
                                    ],
                                    in_=ot[:],
                                )
                            k += 1
                        if batch_store:
                            nc.sync.dma_start(
                                out=y_ap[img, c * 128 : (c + 1) * 128],
                                in_=ob[:],
                            )

            if reps > 1:
                with tc.For_i(0, reps // 2, 1, hint_engines=(mybir.EngineType.PE,)):
                    body()
                    body()
            else:
                body()
            if tok_ap is not None:
                nc.sync.dma_start(out=tok_ap[:, :], in_=bsb[:])
    nc.compile()
    return nc


def _get_nc(reps=1, wdtype=WDTYPE, internal_io=False, store16=STORE16,
            evict=None, batch_store=None):
    evict = EVICT if evict is None else evict
    batch_store = BATCH_STORE if batch_store is None else batch_store
    key = (reps, wdtype, internal_io, store16, evict, batch_store)
    if key not in _cache:
        _cache[key] = _build(reps, wdtype, internal_io, store16, evict,
                             batch_store)
    return _cache[key]


def _prep_inputs(x, weight, bias, wdtype=WDTYPE):
    npdt = np.float16 if wdtype == "float16" else np.float32
    x = np.asarray(x)
    weight = np.ascontiguousarray(weight, dtype=np.float32)
    bias = np.ascontiguousarray(bias, dtype=np.float32)
    # fused pad+cast: one pass over x instead of pad(fp32) then astype
    xpad = np.zeros((B, CI, HP, WP), dtype=npdt)
    xpad[:, :, 1 : H + 1, 1 : W + 1] = x
    # [co, ci, kh, kw] -> [ci, (kh kw), co] flattened to [ci, 9*co]
    wt = np.ascontiguousarray(
        weight.transpose(1, 2, 3, 0).reshape(CI, NTAPS * CO).astype(npdt)
    )
    bt = np.ascontiguousarray(bias.reshape(NCHUNK, 128).T)
    in_maps = [
        {
            "xp": np.ascontiguousarray(xpad[i * BS : (i + 1) * BS]),
            "wt": wt,
            "bt": bt,
        }
        for i in range(N_CORES)
    ]
    return in_maps


def run_sharded(x, weight, bias, trace=False, reps=1, wdtype=WDTYPE):
    """Run on all 8 cores; returns (full_output, BassKernelResults)."""
    from concourse.bass_utils import run_bass_kernel_spmd

    nc = _get_nc(reps, wdtype)
    in_maps = _prep_inputs(x, weight, bias, wdtype)
    res = run_bass_kernel_spmd(nc, in_maps, list(range(N_CORES)), trace=trace)
    y = np.concatenate([res.results[i]["y"] for i in range(N_CORES)], axis=0)
    if y.dtype != np.float32:
        y = y.astype(np.float32)
    return y, res


def kernel(x, weight, bias):
    y, _ = run_sharded(x, weight, bias)
    return y


# revision 13
# speedup vs baseline: 1.5576x; 1.1323x over previous
"""Conv2D 3x3 (stride 1, pad 1) NCHW kernel for Trainium2, 8 NeuronCores.

Reference op: y = conv2d(x[32,128,56,56], w[256,128,3,3]) + b[256]  (fp32)

Strategy:
  - Data-parallel over batch: 4 images per core, 8 cores.
  - Conv as 9 shifted matmuls accumulating in PSUM:
      out[co, (h,w)] = sum_{kh,kw} W[kh,kw][ci,co].T @ xpad[ci, h+kh, w+kw]
    K = ci = 128 sits exactly on the 128 SBUF partitions.
  - Host pre-pads x to 58x58 (zero halo) and pre-transposes the weight to
    [ci, (kh kw), co], so the device never transposes or memsets anything.
  - fp16 operands (host-cast): full PE rate (1 col/cycle), separate
    LDWEIGHTS with FWL that the PE's reorder window hides behind matmuls.
  - Each 8-row output block accumulates in its own PSUM bank (pool bufs=7);
    evictions (bias fused, scalar engine) land in a [128, 3136] SBUF staging
    tile, and the whole (co-chunk, image) output leaves in ONE DMA instead
    of 7 small ones (saves ~8us/rep of per-store overhead at 56 stores/rep).
    Output is staged/stored as fp16 (halves store traffic) and upcast to
    fp32 on the host: adds ~4e-4 max-rel error on top of the ~2e-4 fp16
    matmul error, well inside the 2e-3 gate.
  - Timing loop (reps>1) is unrolled 2x with double-buffered x tiles so
    the next rep's x load overlaps the current rep's matmuls.
"""

import numpy as np

N_CORES = 8
B, CI, H, W = 32, 128, 56, 56
CO = 256
KH = KW = 3
NTAPS = KH * KW
BS = B // N_CORES            # images per core
HP, WP = H + 2, W + 2        # padded image
HB = 8                       # output rows per block
NB = H // HB                 # blocks per image
NTILE = HB * W               # 448 fp32 -> one PSUM bank
NCHUNK = CO // 128           # co chunks of 128 partitions

WDTYPE = "float16"           # matmul operand dtype (host-cast)
STORE16 = True               # store y as fp16, upcast on host (halves store
                             # traffic; adds ~4e-4 max-rel error, well under
                             # the 2e-3 gate)
EVICT = "act"                # "act" | "dve" | "both" | "split" (by image)
BATCH_STORE = True           # stage NB blocks in SBUF, one store per (c,img)
PSUM_BUFS = 7                # PSUM banks in the accumulation pool (7 or 8)
OB_BUFS = 3                  # staging-tile ring depth

_cache = {}


def _build(reps=1, wdtype=WDTYPE, internal_io=False, store16=STORE16,
           evict=None, batch_store=None, psum_bufs=None, ob_bufs=None):
    """Build + compile the SPMD program."""
    import concourse.mybir as mybir
    import concourse.tile as tile
    from concourse import bacc

    evict = EVICT if evict is None else evict
    batch_store = BATCH_STORE if batch_store is None else batch_store
    psum_bufs = PSUM_BUFS if psum_bufs is None else psum_bufs
    ob_bufs = OB_BUFS if ob_bufs is None else ob_bufs

    mmdt = getattr(mybir.dt, wdtype)
    ydt = mybir.dt.float16 if store16 else mybir.dt.float32

    nc = bacc.Bacc("TRN2", target_bir_lowering=False, debug=False)
    if internal_io:
        xp_ap = nc.dram_tensor("xp_i", [BS, CI, HP, WP], mmdt).ap()
        y_ap = nc.dram_tensor("y_i", [BS, CO, H, W], ydt).ap()
        tok_ap = nc.dram_tensor(
            "tok", [128, NCHUNK], mybir.dt.float32, kind="ExternalOutput"
        ).ap()
    else:
        xp_ap = nc.dram_tensor(
            "xp", [BS, CI, HP, WP], mmdt, kind="ExternalInput"
        ).ap()
        y_ap = nc.dram_tensor(
            "y", [BS, CO, H, W], ydt, kind="ExternalOutput"
        ).ap()
        tok_ap = None
    wt_ap = nc.dram_tensor(
        "wt", [CI, NTAPS * CO], mmdt, kind="ExternalInput"
    ).ap()
    bt_ap = nc.dram_tensor(
        "bt", [128, NCHUNK], mybir.dt.float32, kind="ExternalInput"
    ).ap()

    with tile.TileContext(nc) as tc:
        with (
            tc.tile_pool(name="xw", bufs=1) as xw,
            tc.tile_pool(name="out", bufs=ob_bufs if batch_store else 8) as outp,
            tc.tile_pool(name="ps", bufs=psum_bufs, space="PSUM") as ps,
        ):
            wsb = xw.tile([CI, NTAPS * CO], mmdt, tag="w")
            bsb = xw.tile([128, NCHUNK], mybir.dt.float32, tag="b")
            nc.scalar.dma_start(out=wsb[:], in_=wt_ap[:, :])
            nc.scalar.dma_start(out=bsb[:], in_=bt_ap[:, :])

            def evict_one(dst, pt, c, k, img=0):
                if evict == "both":
                    use = "act" if k % 2 else "dve"
                elif evict == "split":
                    # whole images per engine: no two engines write slices
                    # of the same staging tile
                    use = "dve" if img < BS // 2 else "act"
                else:
                    use = evict
                if use == "act":
                    nc.scalar.activation(
                        dst,
                        pt,
                        mybir.ActivationFunctionType.Identity,
                        bias=bsb[:, c : c + 1],
                        scale=1.0,
                    )
                else:
                    nc.vector.tensor_scalar_add(dst, pt, bsb[:, c : c + 1])

            def body():
                xsb = xw.tile([CI, BS * HP * WP], mmdt, tag="x", bufs=2)
                xdma = nc.gpsimd
                head = (HB + 2) * WP
                xflat0 = xp_ap[0].rearrange("c h w -> c (h w)")
                xdma.dma_start(out=xsb[:, 0:head], in_=xflat0[:, 0:head])
                xdma.dma_start(
                    out=xsb[:, head : HP * WP], in_=xflat0[:, head : HP * WP]
                )
                for img in range(1, BS):
                    xdma.dma_start(
                        out=xsb[:, img * HP * WP : (img + 1) * HP * WP],
                        in_=xp_ap[img].rearrange("c h w -> c (h w)")[:, :],
                    )
                xv = xsb[:].rearrange("c (n h w) -> c n h w", n=BS, h=HP)

                k = 0
                for c in range(NCHUNK):
                    for img in range(BS):
                        if batch_store:
                            ob = outp.tile([128, NB * NTILE], ydt, tag="o")
                        for hb in range(NB):
                            pt = ps.tile([128, NTILE], mybir.dt.float32, tag="acc")
                            for kh in range(KH):
                                for kw in range(KW):
                                    tap = kh * KW + kw
                                    r0 = hb * HB + kh
                                    nc.tensor.matmul(
                                        pt[:],
                                        wsb[
                                            :,
                                            tap * CO
                                            + c * 128 : tap * CO
                                            + (c + 1) * 128,
                                        ],
                                        xv[:, img, r0 : r0 + HB, kw : kw + W],
                                        start=(tap == 0),
                                        stop=(tap == NTAPS - 1),
                                    )
                            if batch_store:
                                evict_one(
                                    ob[:, hb * NTILE : (hb + 1) * NTILE],
                                    pt[:], c, k, img,
                                )
                            else:
                                ot = outp.tile([128, NTILE], ydt, tag="o")
                                evict_one(ot[:], pt[:], c, k, img)
                                nc.sync.dma_start(
                                    out=y_ap[
                                        img,
                                        c * 128 : (c + 1) * 128,
                                        hb * HB : (hb + 1) * HB,
                                        :,
                                    ],
                                    in_=ot[:],
                                )
                            k += 1
                        if batch_store:
                            nc.sync.dma_start(
                                out=y_ap[img, c * 128 : (c + 1) * 128],
                                in_=ob[:],
                            )

            if reps > 1:
                with tc.For_i(0, reps // 2, 1, hint_engines=(mybir.EngineType.PE,)):
                    body()
                    body()
            else:
                body()
            if tok_ap is not None:
                nc.sync.dma_start(out=tok_ap[:, :], in_=bsb[:])
    nc.compile()
    return nc


def _get_nc(reps=1, wdtype=WDTYPE, internal_io=False, store16=STORE16,
            evict=None, batch_store=None, psum_bufs=None, ob_bufs=None):
    evict = EVICT if evict is None else evict
    batch_store = BATCH_STORE if batch_store is None else batch_store
    psum_bufs = PSUM_BUFS if psum_bufs is None else psum_bufs
    ob_bufs = OB_BUFS if ob_bufs is None else ob_bufs
    key = (reps, wdtype, internal_io, store16, evict, batch_store,
           psum_bufs, ob_bufs)
    if key not in _cache:
        _cache[key] = _build(reps, wdtype, internal_io, store16, evict,
                             batch_store, psum_bufs, ob_bufs)
    return _cache[key]


def _prep_inputs(x, weight, bias, wdtype=WDTYPE):
    npdt = np.float16 if wdtype == "float16" else np.float32
    x = np.asarray(x)
    weight = np.ascontiguousarray(weight, dtype=np.float32)
    bias = np.ascontiguousarray(bias, dtype=np.float32)
    # fused pad+cast: one pass over x instead of pad(fp32) then astype
    xpad = np.zeros((B, CI, HP, WP), dtype=npdt)
    xpad[:, :, 1 : H + 1, 1 : W + 1] = x
    # [co, ci, kh, kw] -> [ci, (kh kw), co] flattened to [ci, 9*co]
    wt = np.ascontiguousarray(
        weight.transpose(1, 2, 3, 0).reshape(CI, NTAPS * CO).astype(npdt)
    )
    bt = np.ascontiguousarray(bias.reshape(NCHUNK, 128).T)
    in_maps = [
        {
            "xp": np.ascontiguousarray(xpad[i * BS : (i + 1) * BS]),
            "wt": wt,
            "bt": bt,
        }
        for i in range(N_CORES)
    ]
    return in_maps


def run_sharded(x, weight, bias, trace=False, reps=1, wdtype=WDTYPE):
    """Run on all 8 cores; returns (full_output, BassKernelResults)."""
    from concourse.bass_utils import run_bass_kernel_spmd

    nc = _get_nc(reps, wdtype)
    in_maps = _prep_inputs(x, weight, bias, wdtype)
    res = run_bass_kernel_spmd(nc, in_maps, list(range(N_CORES)), trace=trace)
    y = np.concatenate([res.results[i]["y"] for i in range(N_CORES)], axis=0)
    if y.dtype != np.float32:
        y = y.astype(np.float32)
    return y, res


def kernel(x, weight, bias):
    y, _ = run_sharded(x, weight, bias)
    return y
